# revision 2
# baseline (speedup 1.0000x reference)
# Trainium2 Bass kernel for MixedChunkAttention.
#
# Sharding: 8 cores = 4 batches x 2-way tensor-parallel split of INNER
# (E=2048 -> 1024 per core). Each core processes one full batch (the
# cross-chunk kv cumsum stays core-local) and one half of the inner dim;
# the host sums the two partial outputs per batch and adds bout.
#
# Precision scheme (rel_l2 ~= 1e-2 vs f32 reference, gate is 2e-2):
#   x@Wv runs 1-slot fp8e4m3 with the DoubleRow perf mode (2 K-tiles per
#   instruction -> 2x the fp32r matmul rate; measured on hw, the cost
#   model's 4x is wrong). Its quantization error washes out through the
#   attention averaging. The attn@v (quad) and k^T v (kv) matmuls are
#   also fp8 DoubleRow (attn weights, v, lin_k stored fp8).
#   Everything precision-critical (x@Wg, x@Win, attn QK^T, the v_lin
#   state matmul, o@Wout) stays fp32r.
#   Static scales: x_hi*8, Wv*256, lq*4, lk*4, attn*16, Wout/16 (so the
#   out psum holds the final value exactly); o stored as o*16 in f32r.
#
# Per-core dataflow (chunked over G=16 chunks of C=256 positions):
#   xT f32r + x_hi fp8 chunk-pair streamed to SBUF (host pre-quantizes)
#   xhT  = silu(Win^T @ xT)  f32r            [H, 2C]   (PE + ACT)
#   qqT/qkT/lqT/lkT f32r = per-partition affines       (DVE)
#   lk_nat = fp8(transpose(lkT))            [C, H]    (PE f32r transpose)
#   attnT[m,n] = fp8(relu(qkT^T @ qqT)^2 * 16)        (PE f32r, ACT, DVE)
#   v    = fp8(silu(x_hi @ Wv8))  natural [C, E']     (PE fp8 DR + ACT)
#   gT   = silu(Wg^T @ xT)   f32r            [E', 2C]  (PE + ACT)
#   vqlT = St^T @ lqT (f32r) + v^T @ attnT (fp8 DR, psum-fused)
#   oT   = (gT * vqlT) f32r (= o*16)                  (DVE)
#   St  += lk_nat^T @ v  (fp8 DR; St stores kv*4 in f32r)
#   out[c,:] = oT^T @ (Wout/16)  f32r -> bf16 stage -> DMA
#
# The output projection of chunk g is emitted during chunk g+1 so the
# in-order PE queue always has independent work while the oT chain
# (DVE/ACT) drains; kv is emitted before the quad matmuls so the St
# update lands ahead of the oT ops in the DVE queue.

import numpy as np

B, S, D = 4, 4096, 1024
C, H, E = 256, 128, 2048
G = S // C            # 16 chunks
ELOC = E // 2         # per-core inner slice
T = D // 128          # 8 d-tiles
ET = ELOC // 128      # 8 e-tiles
NCORES = 8

SX = 8.0              # x_hi stored scale
SW = 256.0            # Wv stored scale
SA = 16.0             # attn stored scale (sqrt folded into mask values)
SLK = 4.0             # lin_k stored scale
SLQ = 4.0             # lin_q stored scale
# oT holds o*16 (= vql*16 * gate); Wout shipped as Wout/16.

_CACHE = {}


def _build_nc(n_chunks=G, reps=1, with_bv=True):
    import concourse.mybir as mybir
    import concourse.tile as tile
    from concourse import bacc

    F32, F32R, BF16 = mybir.dt.float32, mybir.dt.float32r, mybir.dt.bfloat16
    FP8 = mybir.dt.float8e4
    AF = mybir.ActivationFunctionType
    OP = mybir.AluOpType
    DR = mybir.MatmulPerfMode.DoubleRow

    nc = bacc.Bacc()
    xt_d = nc.declare_dram_parameter("xT", [128, T, S], BF16, isOutput=False)
    xh_d = nc.declare_dram_parameter("xh8", [128, T, S], FP8, isOutput=False)
    wvh_d = nc.declare_dram_parameter("wvh", [128, T, ELOC], FP8, isOutput=False)
    wg_d = nc.declare_dram_parameter("wg", [128, T, ELOC], BF16, isOutput=False)
    win_d = nc.declare_dram_parameter("win", [128, T, H], BF16, isOutput=False)
    wout_d = nc.declare_dram_parameter("wout", [128, ET, D], BF16,
                                       isOutput=False)
    bv_d = nc.declare_dram_parameter("bv", [1, ELOC], F32R, isOutput=False)
    one_d = nc.declare_dram_parameter("ones", [1, 128], F32R, isOutput=False)
    bgt_d = nc.declare_dram_parameter("bgt", [128, ET], F32, isOutput=False)
    aff_d = nc.declare_dram_parameter("aff", [128, 9], F32, isOutput=False)
    msk_d = nc.declare_dram_parameter("masks", [128, 2, C], F32, isOutput=False)
    id_d = nc.declare_dram_parameter("ident", [128, 128], F32R, isOutput=False)
    zs_d = nc.declare_dram_parameter("zeros", [128, ELOC], BF16, isOutput=False)
    out_d = nc.declare_dram_parameter("out", [S, D], BF16, isOutput=True)

    with tile.TileContext(nc) as tc:
        with tc.tile_pool(name="wpool", bufs=1) as wpool, \
             tc.tile_pool(name="spool", bufs=1) as spool, \
             tc.tile_pool(name="xtp", bufs=2) as xtp, \
             tc.tile_pool(name="vp", bufs=2) as vp, \
             tc.tile_pool(name="gp", bufs=1) as gp, \
             tc.tile_pool(name="otp", bufs=2) as otp, \
             tc.tile_pool(name="osp", bufs=2) as osp, \
             tc.tile_pool(name="smallp", bufs=2) as smallp, \
             tc.tile_pool(name="ps512", bufs=4, space="PSUM") as ps512, \
             tc.tile_pool(name="pso", bufs=2, space="PSUM") as pso, \
             tc.tile_pool(name="ps256", bufs=1, space="PSUM") as ps256:

            # ---- persistent tiles ----
            wvh_sb = wpool.tile([128, T, ELOC], FP8, name="wvh_sb")
            wg_et = [wpool.tile([128, T, 128], BF16, name=f"wg_et{i}")
                     for i in range(ET)]
            win_sb = wpool.tile([128, T, H], BF16, name="win_sb")
            wout_sb = wpool.tile([128, ET, D], BF16, name="wout_sb")
            bv_sb = wpool.tile([1, ELOC], F32R, name="bv_sb")
            one_sb = wpool.tile([1, 128], F32R, name="one_sb")
            bgt_sb = wpool.tile([128, ET], F32, name="bgt_sb")
            aff_sb = wpool.tile([128, 9], F32, name="aff_sb")
            msk_sb = wpool.tile([128, 2, C], F32, name="msk_sb")
            ident = wpool.tile([128, 128], F32R, name="ident")
            nc.sync.dma_start(out=win_sb[:], in_=win_d[:])
            for sb, d in ((aff_sb, aff_d), (wvh_sb, wvh_d), (wout_sb, wout_d)):
                nc.scalar.dma_start(out=sb[:], in_=d[:])
            for sb, d in ((msk_sb, msk_d), (ident, id_d), (bgt_sb, bgt_d),
                          (bv_sb, bv_d), (one_sb, one_d)):
                nc.gpsimd.dma_start(out=sb[:], in_=d[:])
            for et in range(ET):
                es = slice(et * 128, (et + 1) * 128)
                nc.gpsimd.dma_start(out=wg_et[et][:], in_=wg_d[:, :, es])

            St = spool.tile([128, ELOC], BF16, name="St")

            import contextlib
            rep_ctx = tc.For_i(0, reps) if reps > 1 else contextlib.nullcontext()
            with rep_ctx:
                nc.sync.dma_start(out=St[:], in_=zs_d[:])
                _chunk_body(nc, tc, n_chunks, with_bv, locals())

    nc.finalize()
    return nc


def _chunk_body(nc, tc, n_chunks, with_bv, env):
    import concourse.mybir as mybir
    F32, F32R, BF16 = mybir.dt.float32, mybir.dt.float32r, mybir.dt.bfloat16
    FP8 = mybir.dt.float8e4
    AF = mybir.ActivationFunctionType
    OP = mybir.AluOpType
    DR = mybir.MatmulPerfMode.DoubleRow
    (xt_d, xh_d, out_d) = (env["xt_d"], env["xh_d"], env["out_d"])
    (wvh_sb, wg_et, win_sb, wout_sb, bv_sb, one_sb, bgt_sb, aff_sb,
     msk_sb, ident, St) = (
        env[k] for k in
        ["wvh_sb", "wg_et", "win_sb", "wout_sb", "bv_sb", "one_sb",
         "bgt_sb", "aff_sb", "msk_sb", "ident", "St"])
    (xtp, vp, gp, otp, osp, smallp, ps512, pso, ps256) = (
        env[k] for k in ["xtp", "vp", "gp", "otp", "osp", "smallp",
                         "ps512", "pso", "ps256"])
    T2 = T // 2
    ZIN = 1.0 / (SX * SW)     # v psum holds z * SX*SW
    assert n_chunks % 2 == 0

    def emit_out(g, oT):
        # ---- output projection f32r (chunk g); psum = out exactly ----
        ostage = osp.tile([128, 2, D], BF16, name="ostage", tag="ostage")
        for ci in range(2):
            cb = slice(ci * 128, (ci + 1) * 128)
            for d2 in range(2):
                ds = slice(d2 * 512, (d2 + 1) * 512)
                o_ps = pso.tile([128, 512], F32, name="o_ps", tag="pso")
                for et in range(ET):
                    nc.tensor.matmul(o_ps[:], oT[:, et, cb],
                                     wout_sb[:, et, ds],
                                     start=(et == 0), stop=(et == ET - 1))
                nc.scalar.activation(ostage[:, ci, ds], o_ps[:], AF.Copy,
                                     bias=0.0)
            nc.sync.dma_start(
                out=out_d[g * C + ci * 128: g * C + (ci + 1) * 128, :],
                in_=ostage[:, ci, :])

    pending = None            # (g, oT) awaiting output projection
    for gp_i in range(n_chunks // 2):
            # ---- load x chunk-pair: f32r transposed + fp8 hi ----
            cs = slice(gp_i * 2 * C, (gp_i + 1) * 2 * C)
            xt = xtp.tile([128, T, 2 * C], BF16, name="xt", tag="xt")
            xht = xtp.tile([128, T, 2 * C], FP8, name="xht", tag="xht")
            nc.sync.dma_start(out=xt[:], in_=xt_d[:, :, cs])
            nc.sync.dma_start(out=xht[:], in_=xh_d[:, :, cs])

            # ---- xh projection f32r ----
            xh2 = smallp.tile([128, 2 * C], F32, name="xh2", tag="xh2", bufs=1)
            xh_ps = ps512.tile([128, 2 * C], F32, name="xh_ps", tag="ps512")
            for t in range(T):
                nc.tensor.matmul(xh_ps[:], win_sb[:, t, :], xt[:, t, :],
                                 start=(t == 0), stop=(t == T - 1))
            nc.scalar.activation(xh2[:], xh_ps[:], AF.Silu,
                                 bias=aff_sb[:, 8:9])

            # ---- gate projection f32r ----
            gT2 = gp.tile([128, ET, 2 * C], BF16, name="gT2", tag="gT2")
            for et in range(ET):
                es = slice(et * 128, (et + 1) * 128)
                g_ps = ps512.tile([128, 2 * C], F32, name="g_ps", tag="ps512")
                for t in range(T):
                    nc.tensor.matmul(g_ps[:], wg_et[et][:, t, :], xt[:, t, :],
                                     start=(t == 0), stop=(t == T - 1))
                nc.scalar.activation(gT2[:, et, :], g_ps[:], AF.Silu,
                                     bias=bgt_sb[:, et:et + 1])

            for gi in range(2):
                g = gp_i * 2 + gi
                co = gi * C                      # column offset into pair slabs
                xh = xh2[:, co:co + C]

                # ---- v natural [C, ELOC], 1-slot fp8 DoubleRow ----
                v_sb = vp.tile([128, 2, ELOC], FP8, name="v_sb", tag="v_sb")
                for ci in range(2):
                    cb = slice(co + ci * 128, co + (ci + 1) * 128)
                    for e2 in range(2):
                        v_ps = ps512.tile([128, 512], F32, name="v_ps",
                                          tag="ps512")
                        if with_bv:
                            nc.tensor.matmul(v_ps[:], one_sb[0:1, :],
                                             bv_sb[0:1, e2 * 512:(e2 + 1) * 512],
                                             start=True, stop=False)
                        for kp in range(T2):
                            ks = slice(2 * kp, 2 * kp + 2)
                            nc.tensor.matmul(
                                v_ps[:], xht[:, ks, cb],
                                wvh_sb[:, ks, e2 * 512:(e2 + 1) * 512],
                                start=(kp == 0 and not with_bv),
                                stop=(kp == T2 - 1), perf_mode=DR)
                        nc.scalar.activation(
                            v_sb[:, ci, e2 * 512:(e2 + 1) * 512], v_ps[:],
                            AF.Silu, bias=0.0, scale=ZIN)

                # ---- affines (lq*4 and lk*4 folded into aff) ----
                qqT = smallp.tile([128, C], BF16, name="qqT", tag="qqT")
                qkT = smallp.tile([128, C], BF16, name="qkT", tag="qkT")
                lqT = smallp.tile([128, C], BF16, name="lqT", tag="lqT")
                lkT = smallp.tile([128, C], F32R, name="lkT", tag="lkT", bufs=1)
                nc.vector.tensor_scalar(out=qqT[:], in0=xh[:],
                                        scalar1=aff_sb[:, 0:1], scalar2=aff_sb[:, 1:2],
                                        op0=OP.mult, op1=OP.add)
                nc.vector.tensor_scalar(out=qkT[:], in0=xh[:],
                                        scalar1=aff_sb[:, 2:3], scalar2=aff_sb[:, 3:4],
                                        op0=OP.mult, op1=OP.add)
                nc.vector.tensor_scalar(out=lqT[:], in0=xh[:],
                                        scalar1=aff_sb[:, 4:5], scalar2=aff_sb[:, 5:6],
                                        op0=OP.mult, op1=OP.add)
                nc.vector.tensor_scalar(out=lkT[:], in0=xh[:],
                                        scalar1=aff_sb[:, 6:7], scalar2=aff_sb[:, 7:8],
                                        op0=OP.mult, op1=OP.add)

                # ---- lk natural via PE transpose (f32r), cvt to fp8 ----
                lkn = smallp.tile([128, 2, H], FP8, name="lkn", tag="lkn")
                tr_ps = ps256.tile([128, 2, 128], F32R, name="tr_ps",
                                   tag="trps")
                for ci in range(2):
                    nc.tensor.matmul(tr_ps[:, ci, :],
                                     lkT[:, ci * 128:(ci + 1) * 128],
                                     ident[:], is_transpose=True,
                                     start=(ci == 0), stop=(ci == 1))
                nc.vector.tensor_copy(lkn[:, 0:2, :], tr_ps[:, 0:2, :])

                # ---- chunk attention attnT[m, n], stored attn*16 ----
                attnT = smallp.tile([128, 2, C], FP8, name="attnT", tag="attnT")
                at_ps = ps256.tile([128, 2, C], F32, name="at_ps", tag="atps")
                for mi in range(2):
                    nc.tensor.matmul(at_ps[:, mi, :],
                                     qkT[:, mi * 128:(mi + 1) * 128],
                                     qqT[:], start=(mi == 0), stop=(mi == 1))
                rt = smallp.tile([128, 2, C], F32, name="rt", tag="rt")
                nc.scalar.activation(rt[:], at_ps[:, 0:2, :], AF.Relu,
                                     bias=0.0)
                nc.vector.tensor_tensor(out=rt[:], in0=rt[:],
                                        in1=msk_sb[:, 0:2, :], op=OP.mult)
                nc.vector.tensor_tensor(out=attnT[:, 0:2, :], in0=rt[:],
                                        in1=rt[:], op=OP.mult)

                # ---- vql = lin (f32r) + quad (fp8 DR); oT = gate * vql ----
                # All lin matmuls first so the PE has cover while the
                # relu/mask/square chain produces attnT.
                oT = otp.tile([128, ET, C], BF16, name="oT", tag="oT")
                vql_pss = []
                for ep in range(ET // 2):
                    vql_ps = ps512.tile([128, 2, C], F32, name="vql_ps",
                                        tag="ps512")
                    vql_pss.append(vql_ps)
                    for j in range(2):
                        et = 2 * ep + j
                        es = slice(et * 128, (et + 1) * 128)
                        nc.tensor.matmul(vql_ps[:, j, :], St[:, es], lqT[:],
                                         start=(j == 0), stop=False)

                # ---- kv state update St += lk_nat^T @ v (fp8 DR) ----
                # Before the quad matmuls so the St add lands ahead of the
                # oT ops in the DVE queue (next chunk's lin matmuls wait).
                for e2 in range(2):
                    kv_ps = ps512.tile([128, 512], F32, name="kv_ps",
                                       tag="ps512")
                    nc.tensor.matmul(kv_ps[:], lkn[:, 0:2, :],
                                     v_sb[:, 0:2, e2 * 512:(e2 + 1) * 512],
                                     start=True, stop=True, perf_mode=DR)
                    nc.vector.tensor_tensor(out=St[:, e2 * 512:(e2 + 1) * 512],
                                            in0=St[:, e2 * 512:(e2 + 1) * 512],
                                            in1=kv_ps[:], op=OP.add)

                for ep in range(ET // 2):
                    vql_ps = vql_pss[ep]
                    for j in range(2):
                        et = 2 * ep + j
                        es = slice(et * 128, (et + 1) * 128)
                        nc.tensor.matmul(vql_ps[:, j, :], v_sb[:, 0:2, es],
                                         attnT[:, 0:2, :],
                                         start=False, stop=(j == 1),
                                         perf_mode=DR)
                    ee = slice(2 * ep, 2 * ep + 2)
                    nc.vector.tensor_tensor(
                        out=oT[:, ee, :], in0=gT2[:, ee, co:co + C],
                        in1=vql_ps[:, 0:2, :], op=OP.mult)

                # ---- previous chunk's output projection (pipelined) ----
                if pending is not None:
                    emit_out(*pending)
                pending = (g, oT)

    if pending is not None:
        emit_out(*pending)


def _get_nc(n_chunks=G, reps=1, with_bv=True):
    key = ("nc", n_chunks, reps, with_bv)
    if key not in _CACHE:
        _CACHE[key] = _build_nc(n_chunks, reps, with_bv)
    return _CACHE[key]


def _fp8(a):
    import ml_dtypes
    return np.asarray(a, np.float32).astype(ml_dtypes.float8_e4m3)


def _prep_inputs(x, Wv, bv, Wg, bg, Win, bin_, Wout, bout,
                 g_qq, b_qq, g_qk, b_qk, g_lq, b_lq, g_lk, b_lk):
    import ml_dtypes
    f = np.float32
    scale = f(E) ** f(0.5)
    tri = np.triu(np.ones((128, 128), f))          # keep p <= col
    sa = f(SA) ** f(0.5)                           # folded into mask values
    masks = np.zeros((128, 512), f)
    masks[:, 0:128] = tri * sa
    masks[:, 128:256] = sa
    masks[:, 256:384] = 0.0
    masks[:, 384:512] = tri * sa
    aff = np.stack([
        g_qq / scale, b_qq / scale, g_qk, b_qk,
        g_lq * SLQ, b_lq * SLQ, g_lk * SLK, b_lk * SLK, bin_],
        axis=1).astype(f)                          # [128, 9]
    ones = np.ones((1, 128), f)
    zeros = np.zeros((128, ELOC), f)

    def dtile(w, n):          # [D, n] -> [128, T, n]
        return np.ascontiguousarray(w.reshape(T, 128, n).transpose(1, 0, 2))

    x = np.asarray(x, f)
    WvH = _fp8(np.asarray(Wv, f) * SW)
    Wg = np.asarray(Wg, f)
    Win = np.asarray(Win, f)
    # oT holds o*16, so ship Wout/16 and the out psum is exact.
    Wout16 = np.asarray(Wout, f) / 16.0

    in_maps = []
    for core in range(NCORES):
        b, h = core // 2, core % 2
        sl = slice(h * ELOC, (h + 1) * ELOC)
        xb = x[b]                                      # [S, D]
        xh8 = _fp8(xb * SX)
        def xtile(a):          # [S, D] -> [128, T, S]
            return np.ascontiguousarray(
                a.T.reshape(T, 128, S).transpose(1, 0, 2))
        wout_l = np.ascontiguousarray(
            Wout16[sl, :].reshape(ET, 128, D).transpose(1, 0, 2))
        in_maps.append({
            "xT": xtile(xb).astype(ml_dtypes.bfloat16),
            "xh8": xtile(xh8),
            "wvh": dtile(WvH[:, sl], ELOC),
            "wg": dtile(Wg[:, sl], ELOC).astype(ml_dtypes.bfloat16),
            "win": dtile(Win, H).astype(ml_dtypes.bfloat16),
            "wout": wout_l.astype(ml_dtypes.bfloat16),
            "bv": (np.asarray(bv, f) * (SX * SW))[sl].reshape(1, ELOC),
            "ones": ones,
            "bgt": np.ascontiguousarray(
                np.asarray(bg, f)[sl].reshape(ET, 128).T),
            "aff": aff,
            "masks": masks.reshape(128, 2, C),
            "ident": np.eye(128, dtype=f),
            "zeros": zeros.astype(ml_dtypes.bfloat16),
        })
    return in_maps


def _run(inputs, trace=False, reps=1, **trace_kw):
    import time
    from concourse.bass_utils import run_bass_kernel_spmd
    with_bv = bool(np.any(np.asarray(inputs["bv"])))
    nc = _get_nc(G, reps, with_bv)
    in_maps = _prep_inputs(**inputs)
    # The axon-tunneled devices occasionally fault transiently
    # (NRT_EXEC_UNIT_UNRECOVERABLE); the pool recovers on a fresh attempt.
    last_exc = None
    for attempt in range(4):
        try:
            res = run_bass_kernel_spmd(nc, in_maps,
                                       core_ids=list(range(NCORES)),
                                       trace=trace, **trace_kw)
            break
        except Exception as e:  # noqa: BLE001
            last_exc = e
            if "UNAVAILABLE" not in str(e) and "unrecoverable" not in str(e):
                raise
            time.sleep(10 * (attempt + 1))
    else:
        raise last_exc
    bout = np.asarray(inputs["bout"], np.float32)
    out = np.zeros((B, S, D), np.float32)
    for core in range(NCORES):
        out[core // 2] += res.results[core]["out"].astype(np.float32)
    out += bout[None, None, :]
    return out, res


def kernel(**inputs) -> np.ndarray:
    inputs = {k: np.asarray(v) for k, v in inputs.items()}
    out, _ = _run(inputs)
    return out

